# revision 13
# baseline (speedup 1.0000x reference)
"""BiRNN encoder-decoder Trainium2 kernel, feature-major layout.

Data-parallel over batch (8 cores x 16 rows). All state is kept
feature-major: h lives in SBUF as [128 (H-chunk), 16 (batch)] fp16 columns,
weights are the PE stationary operand ([k-chunk, n-chunk] tiles of W.T) and
the state is the moving operand, so each recurrent matmul's cost scales with
the 16-wide batch (free size) instead of the 512-wide hidden dim. No
transposes anywhere: the PSUM output [128n, 16b] of one step is exactly the
moving layout the next step needs; tanh evacuates PSUM->SBUF directly.

Decoder feedback is algebraically folded into the layer-0 matmul: with
o0 = lin.h3 + lb and nxt = [o0, x0-o0, x1-x0+o0], layer-0's next-step input
projection W0.nxt becomes A.h3 + B2.[x0;x1] + c0 with A = W0.N.lin (rank-1,
precomputed on host), so the head+feedback hop disappears from the serial
chain; the visible outputs are recovered after the loop by one batched GEMM
over the stored h3 states.
"""
import numpy as np
from contextlib import ExitStack

import concourse.bacc as bacc
import concourse.tile as tile
from concourse import mybir
from concourse.bass_utils import run_bass_kernel_spmd

B, T, IN, H, TGT = 128, 128, 3, 512, 32
NC = 8
BC = B // NC          # 16 batch rows per core
CH = H // 128         # 4 chunks of the hidden dim
F16 = mybir.dt.float16
F32 = mybir.dt.float32
Tanh = mybir.ActivationFunctionType.Tanh

# smalls tile column offsets (fp16 [128, C_SMALL])
B1D0, B1D1 = 0, 512            # enc l1 bias rows (row 0)
DB = 1024                      # dec l1..3 bias rows (row 0), 512 each
C0 = 2560                      # dec l0 const row (row 0)
CS = 3072                      # xin const row [1,2] (row 0)
B2C = 3074                     # dec l0 xin coeffs [2,512] (rows 0-1)
S2C = 3586                     # xin xin-coeffs [2,2] (rows 0-1)
DIN0 = 3588                    # dec l0 t=0 stationary [4,512] (rows 0-3)
XQ = 4100                      # per-core x-init [4,16] rows (x0,x1,1,x2)
LINC = 4116                    # lin head chunks [128,4]
WX2 = 4120                     # xin h3-coeff chunks [128,8]
ONES = 4128                    # all-ones [128,16]
IDC = 4144                     # identity [128,128]
LB = 4272                      # lin_b scalar (row 0)
C_SMALL = 4274

_prog_cache = {}


def _build_program():
    if "nc" in _prog_cache:
        return _prog_cache["nc"]
    nc = bacc.Bacc("TRN2")
    dp = nc.declare_dram_parameter

    # encoder Whh weights are double-fp16 (hi+lo) pairs: fp16 rounding of the
    # recurrent weights is a systematic perturbation that dominates the final
    # error (1.3e-2 alone); the lo-correction matmuls bring it back to ~5e-3.
    whh0_e = dp("whh0", [128, 4 * 2048], F16, isOutput=False)
    xs0_e = dp("xs0", [2, 128, T * 4 * BC], F16, isOutput=False)
    wenc1_e = dp("wenc1", [128, 4 * 2048 + 2 * 4096], F16, isOutput=False)
    wdec_e = dp("wdec", [128, 8 * 2048], F16, isOutput=False)
    smalls_e = dp("smalls", [128, C_SMALL], F16, isOutput=False)
    out_e = dp("out", [1, TGT * BC], F32, isOutput=True)

    SW = T * 4 * BC  # 8192 cols per direction

    with tile.TileContext(nc) as tc, ExitStack() as ctx:
        wpool = ctx.enter_context(tc.tile_pool(name="w", bufs=1))
        hpool = ctx.enter_context(tc.tile_pool(name="h", bufs=1))
        pspool = ctx.enter_context(tc.tile_pool(name="ps", bufs=1, space="PSUM"))

        whh0s = wpool.tile([128, 4 * 2048], F16)   # enc l0 Whh.T (d, hi|lo)
        xs0 = wpool.tile([128, 2 * SW], F16)       # l0 x-proj(+bias), feature-major
        wenc1 = wpool.tile([128, 4 * 2048 + 2 * 4096], F16)  # whh1 hi|lo (2) | wih1 (2)
        wdec = wpool.tile([128, 8 * 2048], F16)    # dwhh(4) | dwihr(3) | A
        smalls = wpool.tile([128, C_SMALL], F16)
        hbuf0 = {d: wpool.tile([128, SW], F16, name=f"hbuf0_{d}") for d in range(2)}
        hbuf3 = wpool.tile([128, TGT * 4 * BC], F16)   # dec l3 states per t

        # critical-path DMAs first (sync queue), bulk weights behind (gpsimd)
        nc.sync.dma_start(whh0s[:], whh0_e[:])
        NXC = 8  # xs chunks per direction
        xcw = SW // NXC
        for i in range(NXC):
            for d in range(2):
                nc.sync.dma_start(xs0[:, d * SW + i * xcw:d * SW + (i + 1) * xcw],
                                  xs0_e[d, :, i * xcw:(i + 1) * xcw])
        nc.gpsimd.dma_start(smalls[:], smalls_e[:])
        nc.gpsimd.dma_start(wenc1[:], wenc1_e[:])
        nc.gpsimd.dma_start(wdec[:], wdec_e[:])

        ident = smalls[:, IDC:IDC + 128]
        ones1 = smalls[0:1, ONES:ONES + 16]

        def mm(ps_ap, lhsT_ap, rhs_ap, start, stop):
            nc.tensor.matmul(ps_ap, lhsT_ap, rhs_ap, start=start, stop=stop)

        # ---- encoder layer 0: fwd (d=0) and bwd (d=1) chains interleaved ----
        # h state for (d, step t) lives at hbuf0[d][:, t*64:(t+1)*64]
        for t in range(T):
            for d in range(2):
                ps = pspool.tile([128, 512], F32, tag=f"psE{d}", name=f"psE{d}", bufs=2)
                xsl = xs0[:, d * SW + t * 64:d * SW + (t + 1) * 64]
                mm(ps[:, 0:64], ident, xsl, True, t == 0)
                if t > 0:
                    hprev = hbuf0[d][:, (t - 1) * 64:t * 64]
                    for kc in range(CH):
                        for nb in range(CH):
                            for part in range(2):  # hi then lo correction
                                o = d * 4096 + part * 2048 + kc * 512 + nb * 128
                                mm(ps[:, 16 * nb:16 * (nb + 1)],
                                   whh0s[:, o:o + 128],
                                   hprev[:, 16 * kc:16 * (kc + 1)],
                                   False, kc == CH - 1 and nb == CH - 1 and part == 1)
                nc.scalar.activation(hbuf0[d][:, t * 64:(t + 1) * 64], ps[:, 0:64], Tanh)

        # ---- encoder layer 1: fused input projection from hbuf0 ----
        WIH1 = 4 * 2048  # offset of wih1 region inside wenc1
        e1h = {}
        for t in range(T):
            for d in range(2):
                f_slot = t if d == 0 else T - 1 - t
                b_slot = T - 1 - t if d == 0 else t
                ps = pspool.tile([128, 512], F32, tag=f"psE{d}", name=f"psF{d}", bufs=2)
                for nb in range(CH):  # bias rows
                    mm(ps[:, 16 * nb:16 * (nb + 1)],
                       smalls[0:1, d * 512 + nb * 128:d * 512 + (nb + 1) * 128],
                       ones1, nb == 0, False)
                for k8 in range(2 * CH):  # input projection (2H contraction)
                    src = hbuf0[0] if k8 < CH else hbuf0[1]
                    slot = f_slot if k8 < CH else b_slot
                    rhs = src[:, slot * 64 + 16 * (k8 % CH):slot * 64 + 16 * (k8 % CH + 1)]
                    for nb in range(CH):
                        mm(ps[:, 16 * nb:16 * (nb + 1)],
                           wenc1[:, WIH1 + d * 4096 + k8 * 512 + nb * 128:WIH1 + d * 4096 + k8 * 512 + (nb + 1) * 128],
                           rhs, False,
                           t == 0 and k8 == 2 * CH - 1 and nb == CH - 1)
                if t > 0:
                    hprev = e1h[d][:, 0:64]
                    for kc in range(CH):
                        for nb in range(CH):
                            for part in range(2):  # hi then lo correction
                                o = d * 4096 + part * 2048 + kc * 512 + nb * 128
                                mm(ps[:, 16 * nb:16 * (nb + 1)],
                                   wenc1[:, o:o + 128],
                                   hprev[:, 16 * kc:16 * (kc + 1)],
                                   False, kc == CH - 1 and nb == CH - 1 and part == 1)
                hnew = hpool.tile([128, 64], F16, tag=f"e1_{d}", name=f"e1_{d}", bufs=2)
                nc.scalar.activation(hnew[:], ps[:, 0:64], Tanh)
                e1h[d] = hnew

        # ---- decoder: 4-layer stack, 32 autoregressive steps ----
        DWIHR = 4 * 2048
        AOFF = 7 * 2048
        hdec = {0: hbuf0[0][:, (T - 1) * 64:T * 64],
                1: hbuf0[1][:, (T - 1) * 64:T * 64],
                2: e1h[0][:, 0:64], 3: e1h[1][:, 0:64]}
        xq = smalls[0:3, XQ:XQ + 16]  # rows (x0, x1, 1)
        for t in range(TGT):
            # layer 0
            ps = pspool.tile([128, 512], F32, tag="psD", name="psD", bufs=2)
            if t == 0:
                for nb in range(CH):
                    mm(ps[:, 16 * nb:16 * (nb + 1)],
                       smalls[0:4, DIN0 + nb * 128:DIN0 + (nb + 1) * 128],
                       smalls[0:4, XQ:XQ + 16], nb == 0, False)
            else:
                h3p = hbuf3[:, (t - 1) * 64:t * 64]
                for kc in range(CH):  # A @ h3
                    for nb in range(CH):
                        mm(ps[:, 16 * nb:16 * (nb + 1)],
                           wdec[:, AOFF + kc * 512 + nb * 128:AOFF + kc * 512 + (nb + 1) * 128],
                           h3p[:, 16 * kc:16 * (kc + 1)],
                           kc == 0 and nb == 0, False)
                for nb in range(CH):  # B2 @ [x0;x1] + c0
                    mm(ps[:, 16 * nb:16 * (nb + 1)],
                       smalls[0:2, B2C + nb * 128:B2C + (nb + 1) * 128],
                       xq[0:2, :], False, False)
                    mm(ps[:, 16 * nb:16 * (nb + 1)],
                       smalls[0:1, C0 + nb * 128:C0 + (nb + 1) * 128],
                       ones1, False, False)
            for kc in range(CH):  # Whh0 @ h0_prev
                for nb in range(CH):
                    mm(ps[:, 16 * nb:16 * (nb + 1)],
                       wdec[:, kc * 512 + nb * 128:kc * 512 + (nb + 1) * 128],
                       hdec[0][:, 16 * kc:16 * (kc + 1)],
                       False, kc == CH - 1 and nb == CH - 1)
            h0 = hpool.tile([128, 64], F16, tag="hd0", name="hd0", bufs=2)
            nc.scalar.activation(h0[:], ps[:, 0:64], Tanh)
            hdec[0] = h0[:]

            # xin update for next step: [x0;x1]_{t+1} from h3_t (issued later,
            # after h3_t exists) -- see below
            # layers 1..3
            for l in range(1, 4):
                ps = pspool.tile([128, 512], F32, tag="psD", name="psD", bufs=2)
                for nb in range(CH):  # bias
                    mm(ps[:, 16 * nb:16 * (nb + 1)],
                       smalls[0:1, DB + (l - 1) * 512 + nb * 128:DB + (l - 1) * 512 + (nb + 1) * 128],
                       ones1, nb == 0, False)
                for kc in range(CH):  # Wih @ h_below
                    for nb in range(CH):
                        mm(ps[:, 16 * nb:16 * (nb + 1)],
                           wdec[:, DWIHR + (l - 1) * 2048 + kc * 512 + nb * 128:DWIHR + (l - 1) * 2048 + kc * 512 + (nb + 1) * 128],
                           hdec[l - 1][:, 16 * kc:16 * (kc + 1)], False, False)
                for kc in range(CH):  # Whh @ h_l_prev
                    for nb in range(CH):
                        mm(ps[:, 16 * nb:16 * (nb + 1)],
                           wdec[:, l * 2048 + kc * 512 + nb * 128:l * 2048 + kc * 512 + (nb + 1) * 128],
                           hdec[l][:, 16 * kc:16 * (kc + 1)],
                           False, kc == CH - 1 and nb == CH - 1)
                if l == 3:
                    nc.scalar.activation(hbuf3[:, t * 64:(t + 1) * 64], ps[:, 0:64], Tanh)
                    hdec[3] = hbuf3[:, t * 64:(t + 1) * 64]
                else:
                    hl = hpool.tile([128, 64], F16, tag=f"hd{l}", name=f"hd{l}", bufs=2)
                    nc.scalar.activation(hl[:], ps[:, 0:64], Tanh)
                    hdec[l] = hl[:]

            if 1 <= t < TGT - 1:
                # xin01_t = Wx2.h3_{t-1} + S2.xin01_{t-1} + cS; h3_{t-1} has
                # been ready since last step, so this chain is off the
                # critical path with a full step of slack.
                px = pspool.tile([128, 512], F32, tag="psX", name="psX", bufs=1)
                for kc in range(CH):
                    mm(px[0:2, 0:16],
                       smalls[:, WX2 + 2 * kc:WX2 + 2 * (kc + 1)],
                       hbuf3[:, (t - 1) * 64 + 16 * kc:(t - 1) * 64 + 16 * (kc + 1)],
                       kc == 0, False)
                mm(px[0:2, 0:16], smalls[0:2, S2C:S2C + 2], xq[0:2, :], False, False)
                mm(px[0:2, 0:16], smalls[0:1, CS:CS + 2], ones1, False, True)
                xnew = hpool.tile([2, 16], F16, tag="xin", name="xin", bufs=2)
                nc.vector.tensor_copy(xnew[:], px[0:2, 0:16])
                xq = xnew[:]

        # ---- head: o0_t = lin.h3_t + lb, all t in one batched group ----
        ph = pspool.tile([128, 512], F32, tag="psX", name="psH", bufs=1)
        for t in range(TGT):
            for kc in range(CH):
                mm(ph[0:1, 16 * t:16 * (t + 1)],
                   smalls[:, LINC + kc:LINC + kc + 1],
                   hbuf3[:, t * 64 + 16 * kc:t * 64 + 16 * (kc + 1)],
                   t == 0 and kc == 0, t == TGT - 1 and kc == CH - 1)
        outt = hpool.tile([1, TGT * BC], F32, tag="out", name="out")
        nc.scalar.activation(outt[:], ph[0:1, 0:TGT * BC],
                             mybir.ActivationFunctionType.Identity,
                             bias=smalls[0:1, LB:LB + 1])
        nc.sync.dma_start(out_e[:], outt[:])

    nc.compile()
    _prog_cache["nc"] = nc
    return nc


def _statT(W):
    """W (N,K), h_new = W @ h -> stationary tile [128, (K//128)*N]:
    chunk kc at cols [kc*N:(kc+1)*N] holds W.T[128*kc:128*(kc+1), :]."""
    W = np.asarray(W, np.float32)
    N, K = W.shape
    WT = np.ascontiguousarray(W.T)
    return WT.reshape(K // 128, 128, N).transpose(1, 0, 2).reshape(128, (K // 128) * N)


def kernel(x, y, enc_Wih0, enc_Whh0, enc_Wih1, enc_Whh1, enc_bih, enc_bhh,
           dec_Wih0, dec_Wihr, dec_Whh, dec_bih, dec_bhh, lin_W, lin_b,
           target_len, teacher_forcing_ratio):
    f, h16 = np.float32, np.float16
    x = np.asarray(x, f)
    enc_Wih0, enc_Whh0 = np.asarray(enc_Wih0, f), np.asarray(enc_Whh0, f)
    enc_Wih1, enc_Whh1 = np.asarray(enc_Wih1, f), np.asarray(enc_Whh1, f)
    enc_bih, enc_bhh = np.asarray(enc_bih, f), np.asarray(enc_bhh, f)
    dec_Wih0, dec_Wihr = np.asarray(dec_Wih0, f), np.asarray(dec_Wihr, f)
    dec_Whh = np.asarray(dec_Whh, f)
    dec_bih, dec_bhh = np.asarray(dec_bih, f), np.asarray(dec_bhh, f)
    lin_W = np.asarray(lin_W, f)
    lb = float(np.asarray(lin_b, f).reshape(()))

    def _hilo(W):
        hi = W.astype(h16).astype(f)
        return [_statT(hi), _statT(W - hi)]

    whh0 = np.concatenate(_hilo(enc_Whh0[0]) + _hilo(enc_Whh0[1]), 1).astype(h16)
    wenc1 = np.concatenate(_hilo(enc_Whh1[0]) + _hilo(enc_Whh1[1])
                           + [_statT(enc_Wih1[d]) for d in range(2)], 1).astype(h16)

    W0, linv = dec_Wih0, lin_W[0]  # (512,3), (512,)
    Nv = np.array([1.0, -1.0, 1.0], f)
    A = np.outer(W0 @ Nv, linv)                      # (512,512)
    b0tot = dec_bih[0] + dec_bhh[0]
    c0 = (W0 @ Nv) * lb + b0tot                      # (512,)
    B2 = np.stack([W0[:, 1] - W0[:, 2], W0[:, 2]])   # (2,512): x0,x1 coeffs
    wdec = np.concatenate([_statT(dec_Whh[l]) for l in range(4)]
                          + [_statT(dec_Wihr[l]) for l in range(3)]
                          + [_statT(A)], 1).astype(h16)

    smalls = np.zeros((128, C_SMALL), f)
    for d in range(2):
        smalls[0, d * 512:(d + 1) * 512] = enc_bih[1, d] + enc_bhh[1, d]
    for l in range(1, 4):
        smalls[0, DB + (l - 1) * 512:DB + l * 512] = dec_bih[l] + dec_bhh[l]
    smalls[0, C0:C0 + 512] = c0
    smalls[0, CS:CS + 2] = [lb, -lb]
    smalls[0:2, B2C:B2C + 512] = B2
    smalls[0:2, S2C:S2C + 2] = np.array([[0, 1], [0, 0]], f)
    din0q = np.zeros((4, 512), f)   # rows match xq rows (x0, x1, 1, x2)
    din0q[0], din0q[1], din0q[3] = W0[:, 0], W0[:, 1], W0[:, 2]
    din0q[2] = b0tot
    smalls[0:4, DIN0:DIN0 + 512] = din0q
    smalls[:, LINC:LINC + 4] = linv.reshape(4, 128).T
    wx2 = np.stack([linv, -linv])                    # (2,512)
    smalls[:, WX2:WX2 + 8] = wx2.T.reshape(4, 128, 2).transpose(1, 0, 2).reshape(128, 8)
    smalls[:, ONES:ONES + 16] = 1.0
    smalls[:, IDC:IDC + 128] = np.eye(128, dtype=f)
    smalls[0, LB] = lb

    nc = _build_program()

    in_maps = []
    for c in range(NC):
        xc = x[c * BC:(c + 1) * BC]  # (16, T, 3)
        xs0 = np.empty((2, 128, T * 4 * BC), h16)
        for d in range(2):
            W = enc_Wih0[d]
            b = enc_bih[0, d] + enc_bhh[0, d]
            proj = np.einsum('bti,hi->tbh', xc, W) + b   # (T, 16, 512)
            if d == 1:
                proj = proj[::-1]
            xs0[d] = proj.reshape(T, BC, 4, 128).transpose(3, 0, 2, 1).reshape(128, T * 4 * BC)
        sm = smalls.copy()
        sm[0:2, XQ:XQ + 16] = xc[:, -1, 0:2].T
        sm[2, XQ:XQ + 16] = 1.0
        sm[3, XQ:XQ + 16] = xc[:, -1, 2]
        in_maps.append({
            "whh0": whh0, "xs0": xs0, "wenc1": wenc1, "wdec": wdec,
            "smalls": sm.astype(h16),
        })
    res = run_bass_kernel_spmd(nc, in_maps, list(range(NC)))
    out = np.stack([res.results[c]["out"].reshape(TGT, BC).T for c in range(NC)])
    return out.reshape(B, TGT, 1).astype(np.float32)


# revision 26
# speedup vs baseline: 1.1147x; 1.1147x over previous
"""BiRNN encoder-decoder Trainium2 kernel, feature-major layout.

Data-parallel over batch (8 cores x 16 rows). All state is kept
feature-major: h lives in SBUF as [128 (H-chunk), 16 (batch)] fp16 columns,
weights are the PE stationary operand ([k-chunk, n-chunk] tiles of W.T) and
the state is the moving operand, so each recurrent matmul's cost scales with
the 16-wide batch (free size) instead of the 512-wide hidden dim. No
transposes anywhere: the PSUM output [128n, 16b] of one step is exactly the
moving layout the next step needs; tanh evacuates PSUM->SBUF directly.

Decoder feedback is algebraically folded into the layer-0 matmul: with
o0 = lin.h3 + lb and nxt = [o0, x0-o0, x1-x0+o0], layer-0's next-step input
projection W0.nxt becomes A.h3 + B2.[x0;x1] + c0 with A = W0.N.lin (rank-1,
precomputed on host), so the head+feedback hop disappears from the serial
chain; the visible outputs are recovered after the loop by one batched GEMM
over the stored h3 states.
"""
import numpy as np
from contextlib import ExitStack

import concourse.bacc as bacc
import concourse.tile as tile
from concourse import mybir
from concourse.bass_utils import run_bass_kernel_spmd

B, T, IN, H, TGT = 128, 128, 3, 512, 32
NC = 8
BC = B // NC          # 16 batch rows per core
CH = H // 128         # 4 chunks of the hidden dim
F16 = mybir.dt.float16
F32 = mybir.dt.float32
Tanh = mybir.ActivationFunctionType.Tanh

# smalls tile column offsets (fp16 [128, C_SMALL])
B1D0, B1D1 = 0, 512            # enc l1 bias rows (row 0)
DB = 1024                      # dec l1..3 bias rows (row 0), 512 each
C0 = 2560                      # dec l0 const row (row 0)
CS = 3072                      # xin const row [1,2] (row 0)
B2C = 3074                     # dec l0 xin coeffs [2,512] (rows 0-1)
S2C = 3586                     # xin xin-coeffs [2,2] (rows 0-1)
DIN0 = 3588                    # dec l0 t=0 stationary [4,512] (rows 0-3)
XQ = 4100                      # per-core x-init [4,16] rows (x0,x1,1,x2)
LINC = 4116                    # lin head chunks [128,4]
WX2 = 4120                     # xin h3-coeff chunks [128,8]
ONES = 4128                    # all-ones [128,16]
IDC = 4144                     # identity [128,128]
LB = 4272                      # lin_b scalar (row 0)
C_SMALL = 4274

_prog_cache = {}


def _build_program():
    if "nc" in _prog_cache:
        return _prog_cache["nc"]
    nc = bacc.Bacc("TRN2")
    dp = nc.declare_dram_parameter

    # encoder Whh weights are double-fp16 (hi+lo) pairs: fp16 rounding of the
    # recurrent weights is a systematic perturbation that dominates the final
    # error (1.3e-2 alone); the lo-correction matmuls bring it back to ~5e-3.
    whh0_e = dp("whh0", [128, 4 * 2048], F16, isOutput=False)
    xs0_e = dp("xs0", [2, 128, T * 4 * BC], F16, isOutput=False)
    wenc1_e = dp("wenc1", [128, 2 * 2048 + 2 * 4096], F16, isOutput=False)
    wdec_e = dp("wdec", [128, 8 * 2048], F16, isOutput=False)
    smalls_e = dp("smalls", [128, C_SMALL], F16, isOutput=False)
    out_e = dp("out", [1, TGT * BC], F32, isOutput=True)

    SW = T * 4 * BC  # 8192 cols per direction

    with tile.TileContext(nc) as tc, ExitStack() as ctx:
        wpool = ctx.enter_context(tc.tile_pool(name="w", bufs=1))
        hpool = ctx.enter_context(tc.tile_pool(name="h", bufs=1))
        pspool = ctx.enter_context(tc.tile_pool(name="ps", bufs=1, space="PSUM"))

        whh0s = wpool.tile([128, 4 * 2048], F16)   # enc l0 Whh.T (d, hi|lo)
        xs0 = wpool.tile([128, 2 * SW], F16)       # l0 x-proj(+bias), feature-major
        wenc1 = wpool.tile([128, 2 * 2048 + 2 * 4096], F16)  # whh1 (2) | wih1 (2)
        wdec = wpool.tile([128, 8 * 2048], F16)    # dwhh(4) | dwihr(3) | A
        smalls = wpool.tile([128, C_SMALL], F16)
        hbuf0 = {d: wpool.tile([128, SW], F16, name=f"hbuf0_{d}") for d in range(2)}
        hbuf3 = wpool.tile([128, TGT * 4 * BC], F16)   # dec l3 states per t

        # xs chunks stream on the sync queue (first chunks small so step 0
        # starts fast); weights go on the gpsimd queue in first-use order so
        # the two queues' transfers overlap.
        bounds = [0, 8, 16, 32, 64, 96, 128]
        for i in range(len(bounds) - 1):
            a, b = bounds[i] * 64, bounds[i + 1] * 64
            for d in range(2):
                nc.sync.dma_start(xs0[:, d * SW + a:d * SW + b],
                                  xs0_e[d, :, a:b])
        nc.gpsimd.dma_start(whh0s[:], whh0_e[:])
        nc.gpsimd.dma_start(smalls[:], smalls_e[:])
        nc.gpsimd.dma_start(wenc1[:], wenc1_e[:])
        nc.gpsimd.dma_start(wdec[:], wdec_e[:])

        ident = smalls[:, IDC:IDC + 128]
        ones1 = smalls[0:1, ONES:ONES + 16]

        def mm(ps_ap, lhsT_ap, rhs_ap, start, stop):
            nc.tensor.matmul(ps_ap, lhsT_ap, rhs_ap, start=start, stop=stop)

        # ---- encoder layer 0: fwd (d=0) and bwd (d=1) chains interleaved ----
        # h state for (d, step t) lives at hbuf0[d][:, t*64:(t+1)*64]
        for t in range(T):
            for d in range(2):
                ps = pspool.tile([128, 512], F32, tag=f"psE{d}", name=f"psE{d}", bufs=2)
                xsl = xs0[:, d * SW + t * 64:d * SW + (t + 1) * 64]
                mm(ps[:, 0:64], ident, xsl, True, t == 0)
                if t > 0:
                    hprev = hbuf0[d][:, (t - 1) * 64:t * 64]
                    for kc in range(CH):
                        for nb in range(CH):
                            for part in range(2):  # hi then lo correction
                                o = d * 4096 + part * 2048 + kc * 512 + nb * 128
                                mm(ps[:, 16 * nb:16 * (nb + 1)],
                                   whh0s[:, o:o + 128],
                                   hprev[:, 16 * kc:16 * (kc + 1)],
                                   False, kc == CH - 1 and nb == CH - 1 and part == 1)
                nc.scalar.activation(hbuf0[d][:, t * 64:(t + 1) * 64], ps[:, 0:64], Tanh)

        # ---- encoder layer 1: fused input projection from hbuf0 ----
        WIH1 = 2 * 2048  # offset of wih1 region inside wenc1
        e1h = {}
        for t in range(T):
            for d in range(2):
                f_slot = t if d == 0 else T - 1 - t
                b_slot = T - 1 - t if d == 0 else t
                ps = pspool.tile([128, 512], F32, tag=f"psE{d}", name=f"psF{d}", bufs=2)
                for nb in range(CH):  # bias rows
                    mm(ps[:, 16 * nb:16 * (nb + 1)],
                       smalls[0:1, d * 512 + nb * 128:d * 512 + (nb + 1) * 128],
                       ones1, nb == 0, False)
                for k8 in range(2 * CH):  # input projection (2H contraction)
                    src = hbuf0[0] if k8 < CH else hbuf0[1]
                    slot = f_slot if k8 < CH else b_slot
                    rhs = src[:, slot * 64 + 16 * (k8 % CH):slot * 64 + 16 * (k8 % CH + 1)]
                    for nb in range(CH):
                        mm(ps[:, 16 * nb:16 * (nb + 1)],
                           wenc1[:, WIH1 + d * 4096 + k8 * 512 + nb * 128:WIH1 + d * 4096 + k8 * 512 + (nb + 1) * 128],
                           rhs, False,
                           t == 0 and k8 == 2 * CH - 1 and nb == CH - 1)
                if t > 0:
                    hprev = e1h[d][:, 0:64]
                    for kc in range(CH):
                        for nb in range(CH):
                            o = d * 2048 + kc * 512 + nb * 128
                            mm(ps[:, 16 * nb:16 * (nb + 1)],
                               wenc1[:, o:o + 128],
                               hprev[:, 16 * kc:16 * (kc + 1)],
                               False, kc == CH - 1 and nb == CH - 1)
                hnew = hpool.tile([128, 64], F16, tag=f"e1_{d}", name=f"e1_{d}", bufs=2)
                nc.scalar.activation(hnew[:], ps[:, 0:64], Tanh)
                e1h[d] = hnew

        # ---- decoder: 4-layer stack, 32 autoregressive steps ----
        DWIHR = 4 * 2048
        AOFF = 7 * 2048
        hdec = {0: hbuf0[0][:, (T - 1) * 64:T * 64],
                1: hbuf0[1][:, (T - 1) * 64:T * 64],
                2: e1h[0][:, 0:64], 3: e1h[1][:, 0:64]}
        xq = smalls[0:3, XQ:XQ + 16]  # rows (x0, x1, 1)
        ph = pspool.tile([128, 512], F32, tag="psH", name="psH", bufs=1)
        for t in range(TGT):
            # layer 0
            ps = pspool.tile([128, 512], F32, tag="psD", name="psD", bufs=2)
            if t == 0:
                for nb in range(CH):
                    mm(ps[:, 16 * nb:16 * (nb + 1)],
                       smalls[0:4, DIN0 + nb * 128:DIN0 + (nb + 1) * 128],
                       smalls[0:4, XQ:XQ + 16], nb == 0, False)
            else:
                # issue order = dependency age: B2/c0 (xin from t-2's chain),
                # whh0 (h0_{t-1}, 3 hops old), A (h3_{t-1}, just produced) --
                # only the A matmuls sit on the serial chain.
                for nb in range(CH):  # B2 @ [x0;x1] + c0
                    mm(ps[:, 16 * nb:16 * (nb + 1)],
                       smalls[0:2, B2C + nb * 128:B2C + (nb + 1) * 128],
                       xq[0:2, :], nb == 0, False)
                    mm(ps[:, 16 * nb:16 * (nb + 1)],
                       smalls[0:1, C0 + nb * 128:C0 + (nb + 1) * 128],
                       ones1, False, False)
                for kc in range(CH):  # Whh0 @ h0_prev
                    for nb in range(CH):
                        mm(ps[:, 16 * nb:16 * (nb + 1)],
                           wdec[:, kc * 512 + nb * 128:kc * 512 + (nb + 1) * 128],
                           hdec[0][:, 16 * kc:16 * (kc + 1)], False, False)
                h3p = hbuf3[:, (t - 1) * 64:t * 64]
                for kc in range(CH):  # A @ h3
                    for nb in range(CH):
                        mm(ps[:, 16 * nb:16 * (nb + 1)],
                           wdec[:, AOFF + kc * 512 + nb * 128:AOFF + kc * 512 + (nb + 1) * 128],
                           h3p[:, 16 * kc:16 * (kc + 1)],
                           False, kc == CH - 1 and nb == CH - 1)
            if t == 0:
                for kc in range(CH):  # Whh0 @ h0_prev (t=0 tail of group)
                    for nb in range(CH):
                        mm(ps[:, 16 * nb:16 * (nb + 1)],
                           wdec[:, kc * 512 + nb * 128:kc * 512 + (nb + 1) * 128],
                           hdec[0][:, 16 * kc:16 * (kc + 1)],
                           False, kc == CH - 1 and nb == CH - 1)
            h0 = hpool.tile([128, 64], F16, tag="hd0", name="hd0", bufs=2)
            nc.scalar.activation(h0[:], ps[:, 0:64], Tanh)
            hdec[0] = h0[:]

            # xin update for next step: [x0;x1]_{t+1} from h3_t (issued later,
            # after h3_t exists) -- see below
            # layers 1..3
            for l in range(1, 4):
                ps = pspool.tile([128, 512], F32, tag="psD", name="psD", bufs=2)
                for nb in range(CH):  # bias (no deps)
                    mm(ps[:, 16 * nb:16 * (nb + 1)],
                       smalls[0:1, DB + (l - 1) * 512 + nb * 128:DB + (l - 1) * 512 + (nb + 1) * 128],
                       ones1, nb == 0, False)
                for kc in range(CH):  # Whh @ h_l_prev (4 hops of slack)
                    for nb in range(CH):
                        mm(ps[:, 16 * nb:16 * (nb + 1)],
                           wdec[:, l * 2048 + kc * 512 + nb * 128:l * 2048 + kc * 512 + (nb + 1) * 128],
                           hdec[l][:, 16 * kc:16 * (kc + 1)], False, False)
                for kc in range(CH):  # Wih @ h_below (same-step: critical)
                    for nb in range(CH):
                        mm(ps[:, 16 * nb:16 * (nb + 1)],
                           wdec[:, DWIHR + (l - 1) * 2048 + kc * 512 + nb * 128:DWIHR + (l - 1) * 2048 + kc * 512 + (nb + 1) * 128],
                           hdec[l - 1][:, 16 * kc:16 * (kc + 1)],
                           False, kc == CH - 1 and nb == CH - 1)
                if l == 3:
                    nc.scalar.activation(hbuf3[:, t * 64:(t + 1) * 64], ps[:, 0:64], Tanh)
                    hdec[3] = hbuf3[:, t * 64:(t + 1) * 64]
                    # head matmuls for this step join one long-lived group in
                    # their own PSUM bank; off the critical chain.
                    for kc in range(CH):
                        mm(ph[0:1, 16 * t:16 * (t + 1)],
                           smalls[:, LINC + kc:LINC + kc + 1],
                           hbuf3[:, t * 64 + 16 * kc:t * 64 + 16 * (kc + 1)],
                           t == 0 and kc == 0, t == TGT - 1 and kc == CH - 1)
                else:
                    hl = hpool.tile([128, 64], F16, tag=f"hd{l}", name=f"hd{l}", bufs=2)
                    nc.scalar.activation(hl[:], ps[:, 0:64], Tanh)
                    hdec[l] = hl[:]

            if 1 <= t < TGT - 1:
                # xin01_t = Wx2.h3_{t-1} + S2.xin01_{t-1} + cS; h3_{t-1} has
                # been ready since last step, so this chain is off the
                # critical path with a full step of slack.
                px = pspool.tile([128, 512], F32, tag="psX", name="psX", bufs=1)
                for kc in range(CH):
                    mm(px[0:2, 0:16],
                       smalls[:, WX2 + 2 * kc:WX2 + 2 * (kc + 1)],
                       hbuf3[:, (t - 1) * 64 + 16 * kc:(t - 1) * 64 + 16 * (kc + 1)],
                       kc == 0, False)
                mm(px[0:2, 0:16], smalls[0:2, S2C:S2C + 2], xq[0:2, :], False, False)
                mm(px[0:2, 0:16], smalls[0:1, CS:CS + 2], ones1, False, True)
                xnew = hpool.tile([2, 16], F16, tag="xin", name="xin", bufs=2)
                nc.vector.tensor_copy(xnew[:], px[0:2, 0:16])
                xq = xnew[:]

        # ---- head output: bias + store ----
        outt = hpool.tile([1, TGT * BC], F32, tag="out", name="out")
        nc.scalar.activation(outt[:], ph[0:1, 0:TGT * BC],
                             mybir.ActivationFunctionType.Identity,
                             bias=smalls[0:1, LB:LB + 1])
        nc.sync.dma_start(out_e[:], outt[:])

    nc.compile()
    _prog_cache["nc"] = nc
    return nc


def _statT(W):
    """W (N,K), h_new = W @ h -> stationary tile [128, (K//128)*N]:
    chunk kc at cols [kc*N:(kc+1)*N] holds W.T[128*kc:128*(kc+1), :]."""
    W = np.asarray(W, np.float32)
    N, K = W.shape
    WT = np.ascontiguousarray(W.T)
    return WT.reshape(K // 128, 128, N).transpose(1, 0, 2).reshape(128, (K // 128) * N)


def kernel(x, y, enc_Wih0, enc_Whh0, enc_Wih1, enc_Whh1, enc_bih, enc_bhh,
           dec_Wih0, dec_Wihr, dec_Whh, dec_bih, dec_bhh, lin_W, lin_b,
           target_len, teacher_forcing_ratio):
    f, h16 = np.float32, np.float16
    x = np.asarray(x, f)
    enc_Wih0, enc_Whh0 = np.asarray(enc_Wih0, f), np.asarray(enc_Whh0, f)
    enc_Wih1, enc_Whh1 = np.asarray(enc_Wih1, f), np.asarray(enc_Whh1, f)
    enc_bih, enc_bhh = np.asarray(enc_bih, f), np.asarray(enc_bhh, f)
    dec_Wih0, dec_Wihr = np.asarray(dec_Wih0, f), np.asarray(dec_Wihr, f)
    dec_Whh = np.asarray(dec_Whh, f)
    dec_bih, dec_bhh = np.asarray(dec_bih, f), np.asarray(dec_bhh, f)
    lin_W = np.asarray(lin_W, f)
    lb = float(np.asarray(lin_b, f).reshape(()))

    def _hilo(W):
        hi = W.astype(h16).astype(f)
        return [_statT(hi), _statT(W - hi)]

    whh0 = np.concatenate(_hilo(enc_Whh0[0]) + _hilo(enc_Whh0[1]), 1).astype(h16)
    wenc1 = np.concatenate([_statT(enc_Whh1[d]) for d in range(2)]
                           + [_statT(enc_Wih1[d]) for d in range(2)], 1).astype(h16)

    W0, linv = dec_Wih0, lin_W[0]  # (512,3), (512,)
    Nv = np.array([1.0, -1.0, 1.0], f)
    A = np.outer(W0 @ Nv, linv)                      # (512,512)
    b0tot = dec_bih[0] + dec_bhh[0]
    c0 = (W0 @ Nv) * lb + b0tot                      # (512,)
    B2 = np.stack([W0[:, 1] - W0[:, 2], W0[:, 2]])   # (2,512): x0,x1 coeffs
    wdec = np.concatenate([_statT(dec_Whh[l]) for l in range(4)]
                          + [_statT(dec_Wihr[l]) for l in range(3)]
                          + [_statT(A)], 1).astype(h16)

    smalls = np.zeros((128, C_SMALL), f)
    for d in range(2):
        smalls[0, d * 512:(d + 1) * 512] = enc_bih[1, d] + enc_bhh[1, d]
    for l in range(1, 4):
        smalls[0, DB + (l - 1) * 512:DB + l * 512] = dec_bih[l] + dec_bhh[l]
    smalls[0, C0:C0 + 512] = c0
    smalls[0, CS:CS + 2] = [lb, -lb]
    smalls[0:2, B2C:B2C + 512] = B2
    smalls[0:2, S2C:S2C + 2] = np.array([[0, 1], [0, 0]], f)
    din0q = np.zeros((4, 512), f)   # rows match xq rows (x0, x1, 1, x2)
    din0q[0], din0q[1], din0q[3] = W0[:, 0], W0[:, 1], W0[:, 2]
    din0q[2] = b0tot
    smalls[0:4, DIN0:DIN0 + 512] = din0q
    smalls[:, LINC:LINC + 4] = linv.reshape(4, 128).T
    wx2 = np.stack([linv, -linv])                    # (2,512)
    smalls[:, WX2:WX2 + 8] = wx2.T.reshape(4, 128, 2).transpose(1, 0, 2).reshape(128, 8)
    smalls[:, ONES:ONES + 16] = 1.0
    smalls[:, IDC:IDC + 128] = np.eye(128, dtype=f)
    smalls[0, LB] = lb

    nc = _build_program()

    in_maps = []
    for c in range(NC):
        xc = x[c * BC:(c + 1) * BC]  # (16, T, 3)
        xs0 = np.empty((2, 128, T * 4 * BC), h16)
        for d in range(2):
            W = enc_Wih0[d]
            b = enc_bih[0, d] + enc_bhh[0, d]
            proj = np.einsum('bti,hi->tbh', xc, W) + b   # (T, 16, 512)
            if d == 1:
                proj = proj[::-1]
            xs0[d] = proj.reshape(T, BC, 4, 128).transpose(3, 0, 2, 1).reshape(128, T * 4 * BC)
        sm = smalls.copy()
        sm[0:2, XQ:XQ + 16] = xc[:, -1, 0:2].T
        sm[2, XQ:XQ + 16] = 1.0
        sm[3, XQ:XQ + 16] = xc[:, -1, 2]
        in_maps.append({
            "whh0": whh0, "xs0": xs0, "wenc1": wenc1, "wdec": wdec,
            "smalls": sm.astype(h16),
        })
    res = run_bass_kernel_spmd(nc, in_maps, list(range(NC)))
    out = np.stack([res.results[c]["out"].reshape(TGT, BC).T for c in range(NC)])
    return out.reshape(B, TGT, 1).astype(np.float32)
